# revision 1
# baseline (speedup 1.0000x reference)
"""Trainium2 Bass kernel for nn_LoraSequential (grouped LoRA + base GEMM).

Computes  y = concat_g[ (x_g @ A_g) @ B_g * 2 ]  +  x @ M   with
BATCH=4096, IN_F=OUT_F=4096, RANK=16, 8 equal segments.

Strategy: pure data parallelism over the 8 NeuronCores. Core g gets
segment g (512 tokens) and its own adapter pair (A_g, B_g) plus a full
copy of M — segments are disjoint so no collectives are needed.

The contraction is split by precision: k-tiles 0..K16-1 run as fp16
matmuls, k-tiles K16..31 (8 of 32) run as fp8e4 DoubleRow matmuls (2x
PE rate, measured 215ns per K=256 k-pair vs 432ns for fp16). fp8
operands are host-prescaled (x*32, M*512, A*512) so fp8 partial sums
carry a 2^14 factor; the x@M tail is accumulated in its own PSUM banks
during a prologue pre-pass, descaled to fp16 in SBUF (o8), and folded
into the fp16 partials at eviction with a DVE add. H = x@A runs
entirely in fp8 DoubleRow; its stationary operand carries the 16 rank
columns replicated at 4 partition-group offsets (stride 32), so H^T
lands pre-replicated and the four K=16 LoRA-correction matmuls of each
o-block issue as ONE concurrent tile_position row-group pack (~230ns
instead of 4x216ns). Max relative error 1.987e-2 (< 2e-2 gate;
bit-deterministic).

Schedule notes: DMA deps are tile-granular, so every transfer that
must unblock compute early gets its own SBUF tile (M8 eighths split by
kp-pair, x8 tail/head, per-chunk xT / M-slab-0 tiles). The DMA engine
starts ~9us in at ~350 GB/s; 12 warm-up matmuls bridge to the first
fp8 data, the pre-pass + H cover the xT/m0 window, and o=0 is consumed
k-chunk-major so it's never gated on a full slab.
"""

import threading

import numpy as np

P = 128          # SBUF partitions / PE array size
BATCH = 4096
IN_F = 4096
OUT_F = 4096
RANK = 16
G = 8            # adapters == cores
SEG = BATCH // G         # 512 tokens per core
KT = IN_F // P           # 32 contraction tiles
KP = KT // 2             # 16 fp8 DoubleRow k-pairs (full depth, for H)
K16 = 24                 # fp16 k-tiles of the main GEMM (0..23)
KH = 12                  # fp8 k-pairs in the x8 "head" (k-tiles 0..23)
KP8 = (KT - K16) // 2    # fp8 k-pairs of the main GEMM tail (4)
TT = SEG // P            # 4 token tiles of 128
NB = 512                 # matmul moving-operand free dim (one PSUM bank)
OB = OUT_F // NB         # 8 output column blocks
QC = 1024                # M8 quarter width (2 o-blocks)
SX = 32.0                # host prescale of x for fp8
SM = 512.0               # host prescale of M for fp8
SA = 512.0               # host prescale of A for fp8
DESCALE = 1.0 / (SX * SM)
# k-chunking of xT / M-slab-0 for the progressive o=0 prologue
KCH = [4, 4, 4, 4, 4, 4]
assert sum(KCH) == K16

_lock = threading.Lock()
_nc = None


def _build_nc():
    import concourse.bacc as bacc
    import concourse.mybir as mybir
    import concourse.tile as tile
    from concourse.bass import ts

    fp16 = mybir.dt.float16
    fp32 = mybir.dt.float32
    fp8 = mybir.dt.float8e4

    nc = bacc.Bacc(None, target_bir_lowering=False)
    # all host-packed to [p, ...] contiguous layouts
    xT = nc.dram_tensor("xT", [P, K16, SEG], fp16, kind="ExternalInput")
    x8t = nc.dram_tensor("x8t", [P, KP8, 2, SEG], fp8, kind="ExternalInput")
    x8h = nc.dram_tensor("x8h", [P, KH, 2, SEG], fp8, kind="ExternalInput")
    # M8 eighths: e = q*2 + h covers o-pair q, kp-pair half h (kp 2h..2h+1)
    M8 = nc.dram_tensor("M8", [8, P, 2, 2, QC], fp8, kind="ExternalInput")
    # A8 stationary: 16 rank cols replicated at partition-group offsets
    # 0/32/64/96 (zeros between) so H^T evicts pre-replicated for the
    # row-group-packed corrections.
    A8 = nc.dram_tensor("A8", [P, KP, 2, P], fp8, kind="ExternalInput")
    B = nc.dram_tensor("B", [RANK, OUT_F], fp16, kind="ExternalInput")
    M16 = nc.dram_tensor("M16", [OB, P, K16, NB], fp16, kind="ExternalInput")
    Y = nc.dram_tensor("Y", [SEG, OUT_F], fp16, kind="ExternalOutput")
    # partition-major view for the combined two-t-tile tail store
    Y_r = Y.rearrange("(t p) (o c) -> p t o c", p=P, c=NB)

    with tile.TileContext(nc) as tc:
        with (
            tc.tile_pool(name="const", bufs=1) as const,
            tc.tile_pool(name="mpool", bufs=2) as mpool,
            tc.tile_pool(name="opool", bufs=4) as opool,
            tc.tile_pool(name="pmain", bufs=7, space="PSUM") as pmain,
            tc.tile_pool(name="ph", bufs=1, space="PSUM") as phpool,
        ):
            warm_in = const.tile([P, NB], fp16)
            nc.gpsimd.memset(warm_in[:, :], 0.0)
            # x8t split in two kp-pair tiles: the first 0.25MB unit
            # unblocks the pre-pass ~0.8us earlier than one 0.5MB tile.
            x8t_s = [const.tile([P, 2, 2, SEG], fp8, name=f"x8t_{h}")
                     for h in range(2)]
            x8h_s = const.tile([P, KH, 2, SEG], fp8)
            # one tile per M8 eighth / xT chunk / m0 chunk: DMA deps
            # are tile-granular, so split tiles make compute eligible
            # as soon as its own bytes land.
            M8_s = [const.tile([P, 2, 2, QC], fp8, name=f"m8e_{e}")
                    for e in range(8)]
            A8_s = const.tile([P, KP, 2, P], fp8)
            o8_s = const.tile([P, TT, OUT_F], fp16)
            xc_s = [const.tile([P, KCH[c], SEG], fp16, name=f"xc_{c}")
                    for c in range(len(KCH))]
            m0_s = [const.tile([P, KCH[c], NB], fp16, name=f"m0c_{c}")
                    for c in range(len(KCH))]
            # B replicated to partition groups 0/32/64/96 (one DMA each)
            # to feed the row-group-packed corrections; HT128 holds H^T
            # replicas at the same offsets (zero rows between, produced
            # by the replicated-A8 matmul — never read).
            B_s = const.tile([P, OUT_F], fp16)
            HT_s = const.tile([P, SEG], fp16)

            # DMA issue order = HWDGE FIFO order: fp8 pre-pass inputs
            # (first two eighths unblock o-pair 0), then A8/B/x8h for H
            # and the corrections, then interleaved xT/m0 k-chunks for
            # the progressive o=0, then M16 slabs in the o-loop.
            nc.sync.dma_start(out=x8t_s[0], in_=x8t[:, 0:2])
            nc.sync.dma_start(out=x8t_s[1], in_=x8t[:, 2:4])
            for e in range(6):
                nc.sync.dma_start(out=M8_s[e], in_=M8[e])
            nc.sync.dma_start(out=A8_s, in_=A8[:, :, :, :])
            for i in range(4):
                nc.sync.dma_start(out=B_s[32 * i : 32 * i + RANK, :], in_=B[:, :])
            for e in range(6, 8):
                nc.sync.dma_start(out=M8_s[e], in_=M8[e])
            kbase = []
            kb = 0
            for c, ck in enumerate(KCH):
                kbase.append(kb)
                nc.sync.dma_start(out=xc_s[c], in_=xT[:, kb : kb + ck, :])
                nc.sync.dma_start(out=m0_s[c], in_=M16[0, :, kb : kb + ck, :])
                kb += ck
            # m1 hoisted ahead of x8h: o=1 is gated by this slab's
            # queue position, while H (using x8h) runs ~6us later.
            m1_s = mpool.tile([P, K16, NB], fp16, tag="mslab", name="mslab_1")
            nc.sync.dma_start(out=m1_s, in_=M16[1])
            nc.sync.dma_start(out=x8h_s, in_=x8h[:, :, :, :])

            # warm-up accumulates into the (later-reused) H bank: its
            # group closes before H's opens, freeing a PSUM bank so the
            # main pool gets 7 buffers.
            ph = phpool.tile([P, SEG], fp32)
            WARM = 12
            for i in range(WARM):
                nc.tensor.matmul(
                    ph,
                    lhsT=warm_in[:, :P],
                    rhs=warm_in,
                    start=(i == 0),
                    stop=(i == WARM - 1),
                )

            # fp8 pre-pass: accumulate the 8-k-tile fp8 tail of x@M for
            # all (t, o) tiles, one o-pair (2 o-blocks) at a time; the
            # stationary x8 slice is reused across the paired o-matmuls
            # of each kp step. Descale 2^-14 on eviction, alternating
            # ACT/DVE so neither engine lags the PE's bank rotation.
            for q in range(4):
                for t in range(TT):
                    p8s = [
                        pmain.tile([P, NB], fp32, tag="ps", name=f"p8_{q}_{t}_{oo}")
                        for oo in range(2)
                    ]
                    for kp in range(KP8):
                        src = M8_s[q * 2 + kp // 2]
                        for oo in range(2):
                            nc.tensor.matmul(
                                p8s[oo],
                                lhsT=x8t_s[kp // 2][:, kp % 2, :, ts(t, P)],
                                rhs=src[:, kp % 2, :, ts(oo, NB)],
                                start=(kp == 0),
                                stop=(kp == KP8 - 1),
                                perf_mode=mybir.MatmulPerfMode.DoubleRow,
                            )
                    # alternate eviction engines: ACT alone lags the
                    # 6.9us/q PE pace and stalls the bank rotation.
                    nc.scalar.mul(o8_s[:, t, ts(q * 2, NB)], p8s[0], DESCALE)
                    nc.vector.tensor_scalar_mul(
                        o8_s[:, t, ts(q * 2 + 1, NB)], p8s[1], DESCALE
                    )

            def packed_corr(ps_tiles, o, idxs=range(TT)):
                # K=16 corrections in one concurrent row-group pack:
                # group i multiplies H^T tokens t=i (partitions 32i+r)
                # by B (same partitions) into bank ps_tiles[i].
                for i in idxs:
                    nc.tensor.matmul(
                        ps_tiles[i],
                        lhsT=HT_s[32 * i : 32 * i + RANK, ts(i, P)],
                        rhs=B_s[32 * i : 32 * i + RANK, ts(o, NB)],
                        start=False,
                        stop=True,
                        tile_position=(32 * i, 0),
                    )

            def evict(ps, t, o, split):
                o_s = opool.tile([P, NB], fp16, tag="osb", name=f"osb_{o}_{t}")
                if split:
                    # Last tile is on the critical tail: split the
                    # eviction so the first half's store overlaps the
                    # second half's PSUM->SBUF add.
                    HB = NB // 2
                    for h in range(2):
                        nc.vector.tensor_tensor(
                            o_s[:, ts(h, HB)],
                            ps[:, ts(h, HB)],
                            o8_s[:, t, o * NB + h * HB : o * NB + (h + 1) * HB],
                            mybir.AluOpType.add,
                        )
                        nc.sync.dma_start(
                            out=Y[ts(t, P), o * NB + h * HB : o * NB + (h + 1) * HB],
                            in_=o_s[:, ts(h, HB)],
                        )
                else:
                    nc.vector.tensor_tensor(
                        o_s, ps, o8_s[:, t, ts(o, NB)], mybir.AluOpType.add
                    )
                    nc.sync.dma_start(out=Y[ts(t, P), ts(o, NB)], in_=o_s)

            # o=0 consumed k-chunk-major so each chunk pair is eligible
            # as it lands; packed corrections after H.
            ps0 = [
                pmain.tile([P, NB], fp32, tag="ps", name=f"ps_0_{t}")
                for t in range(TT)
            ]
            for c, ck in enumerate(KCH):
                for t in range(TT):
                    for k in range(ck):
                        nc.tensor.matmul(
                            ps0[t],
                            lhsT=xc_s[c][:, k, ts(t, P)],
                            rhs=m0_s[c][:, k, :],
                            start=(c == 0 and k == 0),
                            stop=False,
                        )
            # H = x @ A over all 32 k-tiles, fp8 DoubleRow with the
            # 4-replica stationary (deferred until after o=0's chains —
            # it's only needed by the correction packs); fold the LoRA
            # *2.0 and the fp8 descale into the eviction.
            for kp in range(KP):
                hsrc = (x8h_s[:, kp, :, :] if kp < KH
                        else x8t_s[(kp - KH) // 2][:, (kp - KH) % 2, :, :])
                nc.tensor.matmul(
                    ph,
                    lhsT=A8_s[:, kp, :, :],
                    rhs=hsrc,
                    start=(kp == 0),
                    stop=(kp == KP - 1),
                    perf_mode=mybir.MatmulPerfMode.DoubleRow,
                )
            nc.scalar.mul(HT_s[:, :], ph, 2.0 / (SX * SA))
            packed_corr(ps0, 0)
            for t in range(TT):
                evict(ps0[t], t, 0, split=False)

            for o in range(1, OB):
                if o == 1:
                    m_s = m1_s
                else:
                    m_s = mpool.tile([P, K16, NB], fp16, tag="mslab",
                                     name=f"mslab_{o}")
                    nc.sync.dma_start(out=m_s, in_=M16[o])
                pso = [
                    pmain.tile([P, NB], fp32, tag="ps", name=f"ps_{o}_{t}")
                    for t in range(TT)
                ]

                def chain(t):
                    for c, ck in enumerate(KCH):
                        for k in range(ck):
                            nc.tensor.matmul(
                                pso[t],
                                lhsT=xc_s[c][:, k, ts(t, P)],
                                rhs=m_s[:, kbase[c] + k, :],
                                start=(c == 0 and k == 0),
                                stop=False,
                            )

                if o < OB - 1:
                    for t in range(TT):
                        chain(t)
                    packed_corr(pso, o)
                    for t in range(TT):
                        evict(pso[t], t, o, split=False)
                else:
                    # Last o-block: 2+2 corr packs so the first pair's
                    # evictions overlap the remaining chains; the final
                    # two tiles stage into one buffer and store with a
                    # single DMA (one ~600ns sync-issue on the tail
                    # instead of three).
                    chain(0)
                    chain(1)
                    packed_corr(pso, o, idxs=(0, 1))
                    evict(pso[0], 0, o, split=False)
                    evict(pso[1], 1, o, split=False)
                    chain(2)
                    chain(3)
                    packed_corr(pso, o, idxs=(2, 3))
                    o_d = opool.tile([P, 2, NB], fp16, tag="odual",
                                     name="osb_tail")
                    for h, t in enumerate((2, 3)):
                        nc.vector.tensor_tensor(
                            o_d[:, h, :], pso[t], o8_s[:, t, ts(o, NB)],
                            mybir.AluOpType.add,
                        )
                    nc.sync.dma_start(out=Y_r[:, 2:4, o, :], in_=o_d)
    nc.finalize()
    return nc


def get_nc():
    global _nc
    with _lock:
        if _nc is None:
            _nc = _build_nc()
        return _nc


def make_in_maps(x, lora_A, lora_B, M):
    import ml_dtypes

    E4 = ml_dtypes.float8_e4m3
    x2 = np.ascontiguousarray(np.asarray(x, dtype=np.float16).reshape(BATCH, IN_F))
    lora_A = np.asarray(lora_A, dtype=np.float16)
    lora_B = np.asarray(lora_B, dtype=np.float16)
    M = np.ascontiguousarray(np.asarray(M, dtype=np.float16))
    K16R = K16 * P  # fp16 k rows
    # fp16 head of M packed slab-contiguous: [o, p, k, c]
    M16 = np.ascontiguousarray(
        M[:K16R].reshape(K16, P, OB, NB).transpose(2, 1, 0, 3)
    )
    # fp8 tail of M in eighths [q*2+h, p, kp2, i, qc], prescaled by SM
    M8 = np.ascontiguousarray(
        (M[K16R:].astype(np.float32) * np.float32(SM))
        .astype(E4)
        .reshape(2, 2, 2, P, 4, QC)          # (h, kp2, i, p, q, c)
        .transpose(4, 0, 3, 1, 2, 5)         # (q, h, p, kp2, i, c)
        .reshape(8, P, 2, 2, QC)
    )
    in_maps = []
    for g in range(G):
        seg = x2[g * SEG : (g + 1) * SEG]
        segT32 = seg.astype(np.float32).T  # [IN_F, SEG]
        xT = np.ascontiguousarray(
            seg[:, :K16R].T.reshape(K16, P, SEG).transpose(1, 0, 2)
        )
        x8h = np.ascontiguousarray(
            (segT32[:K16R] * np.float32(SX))
            .astype(E4)
            .reshape(KH, 2, P, SEG)
            .transpose(2, 0, 1, 3)
        )
        x8t = np.ascontiguousarray(
            (segT32[K16R:] * np.float32(SX))
            .astype(E4)
            .reshape(KP8, 2, P, SEG)
            .transpose(2, 0, 1, 3)
        )
        a8core = (
            (lora_A[g].astype(np.float32) * np.float32(SA))
            .astype(E4)
            .reshape(KP, 2, P, RANK)
            .transpose(2, 0, 1, 3)
        )  # [P, KP, 2, RANK]
        a8 = np.zeros((P, KP, 2, P), dtype=E4)
        for i in range(4):
            a8[:, :, :, 32 * i : 32 * i + RANK] = a8core
        in_maps.append(
            {
                "xT": xT,
                "x8t": x8t,
                "x8h": x8h,
                "A8": np.ascontiguousarray(a8),
                "B": np.ascontiguousarray(lora_B[g]),
                "M16": M16,
                "M8": M8,
            }
        )
    return in_maps


def kernel(x, lora_A, lora_B, M):
    from concourse.bass_utils import run_bass_kernel_spmd

    nc = get_nc()
    in_maps = make_in_maps(x, lora_A, lora_B, M)
    res = run_bass_kernel_spmd(nc, in_maps, core_ids=list(range(G))).results
    y = np.concatenate([r["Y"] for r in res], axis=0)
    return y.reshape(BATCH, 1, OUT_F)

